# revision 3
# baseline (speedup 1.0000x reference)
"""H2O Llama attention (streaming) on 8 TRN2 NeuronCores.

Tensor-parallel over heads: each core computes 4 of 32 heads for the
Q/K/V projections + attention, all-gathers the per-head attention
outputs, then computes a 512-column slice of the output projection.

Layouts on device (per core, fp32 everywhere):
  xt    [4096, 2048]  hidden_states^T (replicated)
  wqt/wkt/wvt [4096, 512]  W[c*512:(c+1)*512, :].T  (column-sharded)
  wot   [4096, 512]   Wo[c*512:(c+1)*512, :].T      (row-sharded)
  Q^T/K^T kept as [128 d, 2048 q] per head; V natural [2048 kpos, 512 d].
  Scores computed transposed: S^T[kpos, q] so AV needs no transposes.
"""

import contextlib
import ctypes
import math
import sys
import types

import numpy as np

# ---- NTFF profile hook shim (lets BASS_TRACE=1 work in this container) ----
if "antenv.axon_hooks" not in sys.modules:
    def _mk_hook():
        try:
            lib = ctypes.CDLL("/opt/axon/libaxon_pjrt.so")
        except OSError:
            return None
        if not hasattr(lib, "axon_start_nrt_profile"):
            return None
        lib.axon_start_nrt_profile.argtypes = [ctypes.POINTER(ctypes.c_int64), ctypes.c_size_t]
        lib.axon_start_nrt_profile.restype = ctypes.c_int64
        lib.axon_stop_nrt_profile.argtypes = [ctypes.c_char_p]
        lib.axon_stop_nrt_profile.restype = ctypes.c_int64

        @contextlib.contextmanager
        def _hook(output_dir, device_ids=None):
            import jax
            jax.devices()
            if device_ids:
                ids = (ctypes.c_int64 * len(device_ids))(*device_ids)
                rc = lib.axon_start_nrt_profile(ids, len(device_ids))
            else:
                rc = lib.axon_start_nrt_profile(None, 0)
            if rc != 0:
                raise RuntimeError(f"axon_start_nrt_profile rc={rc}")
            try:
                yield
            finally:
                n = lib.axon_stop_nrt_profile(str(output_dir).encode())
                print(f"profile: {n} file(s) -> {output_dir}", file=sys.stderr)

        return _hook

    _m = types.ModuleType("antenv.axon_hooks")
    _hk = _mk_hook()
    _m.get_axon_ntff_profile_hook = lambda: _hk
    _m.set_axon_ntff_profile_hook = lambda h: None
    sys.modules["antenv.axon_hooks"] = _m
# ---------------------------------------------------------------------------

import concourse.bass as bass  # noqa: E402
import concourse.mybir as mybir  # noqa: E402
import concourse.tile as tile  # noqa: E402
from concourse import bacc  # noqa: E402
from concourse.bass_utils import run_bass_kernel_spmd  # noqa: E402
from concourse.masks import make_identity  # noqa: E402

F32 = mybir.dt.float32
NCORES = 8
S, H = 2048, 4096
NH, HD = 32, 128
HPC = 4                # heads per core
DLOC = HPC * HD        # 512 local features
SINK, RECENT = 4, 204  # int(2048*0.1) = 204
EV = SINK + RECENT     # 208
QB = 512               # q block
NQB = S // QB          # 4
CCH = H // 128         # 32 contraction chunks
SCALE = 1.0 / math.sqrt(HD)

_CACHE = {}
LAST_RESULTS = None


def _build():
    nc = bacc.Bacc("TRN2", target_bir_lowering=False, debug=False, num_devices=NCORES)

    xt = nc.declare_dram_parameter("xt", [H, S], F32, isOutput=False)
    wqt = nc.declare_dram_parameter("wqt", [H, DLOC], F32, isOutput=False)
    wkt = nc.declare_dram_parameter("wkt", [H, DLOC], F32, isOutput=False)
    wvt = nc.declare_dram_parameter("wvt", [H, DLOC], F32, isOutput=False)
    wot = nc.declare_dram_parameter("wot", [H, DLOC], F32, isOutput=False)
    cost = nc.declare_dram_parameter("cost", [HD, S], F32, isOutput=False)
    sinn = nc.declare_dram_parameter("sinn", [HD, S], F32, isOutput=False)
    maskc = nc.declare_dram_parameter("maskc", [128, 896], F32, isOutput=False)

    out = nc.declare_dram_parameter("out", [S, DLOC], F32, isOutput=True)
    kev = nc.declare_dram_parameter("kev", [HPC, EV, HD], F32, isOutput=True)
    vev = nc.declare_dram_parameter("vev", [HPC, EV, HD], F32, isOutput=True)

    with tile.TileContext(nc) as tc, contextlib.ExitStack() as ctx:
        persist = ctx.enter_context(tc.tile_pool(name="persist", bufs=1))
        pk = ctx.enter_context(tc.tile_pool(name="kstore", bufs=1))
        pv = ctx.enter_context(tc.tile_pool(name="vstore", bufs=1))
        ps = ctx.enter_context(tc.tile_pool(name="ps", bufs=1, space="PSUM"))
        dram = ctx.enter_context(tc.tile_pool(name="dram", bufs=1, space="DRAM"))

        cos_sb = persist.tile([HD, S], F32, tag="cos", name="cos_sb")
        sin_sb = persist.tile([HD, S], F32, tag="sin", name="sin_sb")
        msk_sb = persist.tile([128, 896], F32, tag="msk", name="msk_sb")
        ones_col = persist.tile([128, 1], F32, tag="onec", name="ones_col")
        ones_row = persist.tile([1, 128], F32, tag="oner", name="ones_row")
        ident = persist.tile([128, 128], F32, tag="ident", name="ident")
        nc.sync.dma_start(cos_sb[:], cost[:])
        nc.sync.dma_start(sin_sb[:], sinn[:])
        nc.sync.dma_start(msk_sb[:], maskc[:])
        nc.gpsimd.memset(ones_col[:], 1.0)
        nc.gpsimd.memset(ones_row[:], 1.0)
        make_identity(nc, ident[:])

        kst = [pk.tile([HD, S], F32, tag=f"k{h}", name=f"kst{h}") for h in range(HPC)]
        vst = [pv.tile([128, DLOC], F32, tag=f"v{i}", name=f"vst{i}") for i in range(S // 128)]

        ag_in = dram.tile([DLOC, S], F32, tag="agin", name="ag_in")
        ag_out = dram.tile([H, S], F32, tag="agout", addr_space="Shared", name="ag_out")

        def psum(tag):
            return ps.tile([128, QB], F32, tag=tag, bufs=1, name=f"ps_{tag}")

        with tc.tile_pool(name="work", bufs=1) as work:
            def wtile(tag, bufs=3):
                return work.tile([128, QB], F32, tag=tag, bufs=bufs, name=f"w_{tag}")

            for j in range(NQB):
                qsl = slice(QB * j, QB * (j + 1))
                # ---- QKV projections for this q-block ----
                xts = []
                for c in range(CCH):
                    t = work.tile([128, QB], F32, tag=f"x{c}", bufs=1, name=f"xt{c}")
                    nc.sync.dma_start(t[:], xt[128 * c:128 * (c + 1), qsl])
                    xts.append(t)
                qp = [psum(f"g{h}") for h in range(HPC)]
                kp = [psum(f"g{4 + h}") for h in range(HPC)]
                for c in range(CCH):
                    wq = wtile("wq")
                    nc.sync.dma_start(wq[:], wqt[128 * c:128 * (c + 1), :])
                    wk = wtile("wk")
                    nc.sync.dma_start(wk[:], wkt[128 * c:128 * (c + 1), :])
                    st, sp = (c == 0), (c == CCH - 1)
                    for h in range(HPC):
                        hs = slice(128 * h, 128 * (h + 1))
                        nc.tensor.matmul(qp[h][:], wq[:, hs], xts[c][:], start=st, stop=sp)
                        nc.tensor.matmul(kp[h][:], wk[:, hs], xts[c][:], start=st, stop=sp)
                # rope: dest = x*cos + shuffle(x)*sinN
                qt_sb = []
                for h in range(HPC):
                    t1 = wtile("rt1", 2)
                    t2 = wtile("rt2", 2)
                    qd = wtile(f"q{h}", 2)
                    nc.vector.tensor_mul(t1[:], qp[h][:], cos_sb[:, qsl])
                    nc.vector.tensor_mul(t2[0:64, :], qp[h][64:128, :], sin_sb[0:64, qsl])
                    nc.vector.tensor_mul(t2[64:128, :], qp[h][0:64, :], sin_sb[64:128, qsl])
                    nc.vector.tensor_add(qd[:], t1[:], t2[:])
                    qt_sb.append(qd)
                    t1k = wtile("rt1", 2)
                    t2k = wtile("rt2", 2)
                    nc.vector.tensor_mul(t1k[:], kp[h][:], cos_sb[:, qsl])
                    nc.vector.tensor_mul(t2k[0:64, :], kp[h][64:128, :], sin_sb[0:64, qsl])
                    nc.vector.tensor_mul(t2k[64:128, :], kp[h][0:64, :], sin_sb[64:128, qsl])
                    nc.vector.tensor_add(kst[h][:, qsl], t1k[:], t2k[:])
                # V projection (natural layout)
                vp = [psum(f"g{kk}") for kk in range(HPC)]
                for c in range(CCH):
                    wv = wtile("wv")
                    nc.sync.dma_start(wv[:], wvt[128 * c:128 * (c + 1), :])
                    st, sp = (c == 0), (c == CCH - 1)
                    for kk in range(4):
                        ks = slice(128 * kk, 128 * (kk + 1))
                        nc.tensor.matmul(vp[kk][:], xts[c][:, ks], wv[:], start=st, stop=sp)
                for kk in range(4):
                    nc.vector.tensor_copy(vst[4 * j + kk][:], vp[kk][:])

                # ---- attention for this q-block ----
                nm = 4 * (j + 1)
                for h in range(HPC):
                    hs = slice(128 * h, 128 * (h + 1))
                    op = psum("g5")
                    rp = psum("g7")
                    for m in range(nm):
                        sp_ = psum("g4" if m % 2 == 0 else "g6")
                        nc.tensor.matmul(
                            sp_[:], kst[h][:, 128 * m:128 * (m + 1)], qt_sb[h][:],
                            start=True, stop=True)
                        p = wtile("p", 2)
                        nc.scalar.activation(p[:], sp_[:],
                                             mybir.ActivationFunctionType.Exp,
                                             scale=SCALE)
                        r = m - 4 * j
                        if r >= 0:
                            nc.vector.tensor_mul(
                                p[:], p[:], msk_sb[:, 384 - 128 * r:896 - 128 * r])
                        st, spn = (m == 0), (m == nm - 1)
                        nc.tensor.matmul(op[:], vst[m][:, hs], p[:], start=st, stop=spn)
                        nc.tensor.matmul(rp[0:1, :], ones_col[:], p[:], start=st, stop=spn)
                    rec = work.tile([1, QB], F32, tag="rec", bufs=2, name="rec")
                    nc.vector.reciprocal(rec[:], rp[0:1, :])
                    bp = psum("g4")
                    nc.tensor.matmul(bp[:], ones_row[:], rec[:], start=True, stop=True)
                    bsb = wtile("bsb", 2)
                    nc.scalar.copy(bsb[:], bp[:])
                    ot = wtile("ot", 2)
                    nc.vector.tensor_mul(ot[:], op[:], bsb[:])
                    nc.sync.dma_start(ag_in[128 * h:128 * (h + 1), qsl], ot[:])

            # ---- evicted K/V outputs ----
            for h in range(HPC):
                hs = slice(128 * h, 128 * (h + 1))
                for (c0, c1, r0) in ((0, SINK, 0), (1844, 1972, SINK), (1972, 2048, 132)):
                    n = c1 - c0
                    tp = psum("g5")
                    nc.tensor.transpose(tp[0:n, 0:128], kst[h][:, c0:c1], ident[:])
                    evt = work.tile([128, 128], F32, tag="ev", bufs=2, name="evt")
                    nc.vector.tensor_copy(evt[0:n, :], tp[0:n, 0:128])
                    nc.sync.dma_start(kev[h, r0:r0 + n, :], evt[0:n, :])
                nc.sync.dma_start(vev[h, 0:SINK, :], vst[0][0:SINK, hs])
                nc.sync.dma_start(vev[h, SINK:80, :], vst[14][52:128, hs])
                nc.sync.dma_start(vev[h, 80:EV, :], vst[15][:, hs])

        # ---- all-gather attention outputs ----
        nc.gpsimd.collective_compute(
            "AllGather", mybir.AluOpType.bypass,
            replica_groups=[list(range(NCORES))],
            ins=[ag_in[:].opt()], outs=[ag_out[:].opt()])

        # ---- output projection: out[q, :512] ----
        with tc.tile_pool(name="oproj", bufs=1) as op_pool:
            wos = []
            for c in range(CCH):
                t = op_pool.tile([128, DLOC], F32, tag=f"wo{c}", bufs=1, name=f"wo{c}")
                nc.sync.dma_start(t[:], wot[128 * c:128 * (c + 1), :])
                wos.append(t)
            for mq in range(4):
                o2 = [psum(f"g{sub}") for sub in range(4)]
                for c in range(CCH):
                    agt = op_pool.tile([128, QB], F32, tag="agt", bufs=3, name="agt")
                    nc.sync.dma_start(
                        agt[:], ag_out[128 * c:128 * (c + 1), QB * mq:QB * (mq + 1)])
                    st, sp = (c == 0), (c == CCH - 1)
                    for sub in range(4):
                        nc.tensor.matmul(o2[sub][:], agt[:, 128 * sub:128 * (sub + 1)],
                                         wos[c][:], start=st, stop=sp)
                for sub in range(4):
                    osb = op_pool.tile([128, DLOC], F32, tag="osb", bufs=2, name="osb")
                    nc.vector.tensor_copy(osb[:], o2[sub][:])
                    nc.sync.dma_start(out[QB * mq + 128 * sub:QB * mq + 128 * (sub + 1), :],
                                      osb[:])

    nc.compile()
    return nc


def kernel(hidden_states, Wq, Wk, Wv, Wo, position_ids):
    global LAST_RESULTS
    if "nc" not in _CACHE:
        _CACHE["nc"] = _build()
    nc = _CACHE["nc"]

    x = np.asarray(hidden_states, np.float32)[0]
    XT = np.ascontiguousarray(x.T)
    pos = np.asarray(position_ids).reshape(-1).astype(np.float64)
    inv = 1.0 / (10000.0 ** (np.arange(0, HD, 2, dtype=np.float64) / HD))
    fr = np.outer(inv, pos)
    cosT = np.concatenate([np.cos(fr), np.cos(fr)], 0).astype(np.float32)
    sinN = np.concatenate([-np.sin(fr), np.sin(fr)], 0).astype(np.float32)
    maskc = (np.arange(128)[:, None] <= (np.arange(896)[None, :] - 384)).astype(np.float32)

    Wq = np.asarray(Wq, np.float32)
    Wk = np.asarray(Wk, np.float32)
    Wv = np.asarray(Wv, np.float32)
    Wo = np.asarray(Wo, np.float32)

    in_maps = []
    for c in range(NCORES):
        sl = slice(DLOC * c, DLOC * (c + 1))
        in_maps.append({
            "xt": XT,
            "wqt": np.ascontiguousarray(Wq[sl, :].T),
            "wkt": np.ascontiguousarray(Wk[sl, :].T),
            "wvt": np.ascontiguousarray(Wv[sl, :].T),
            "wot": np.ascontiguousarray(Wo[sl, :].T),
            "cost": cosT,
            "sinn": sinN,
            "maskc": maskc,
        })

    res = run_bass_kernel_spmd(nc, in_maps, list(range(NCORES)))
    LAST_RESULTS = res

    attn = np.concatenate([res.results[c]["out"] for c in range(NCORES)], axis=1)[None]
    k_ev = np.concatenate([res.results[c]["kev"] for c in range(NCORES)], axis=0)[None]
    v_ev = np.concatenate([res.results[c]["vev"] for c in range(NCORES)], axis=0)[None]
    return attn, k_ev, v_ev
